# revision 22
# baseline (speedup 1.0000x reference)
"""DDSCTransformer Trainium2 kernel.

Sharding: data-parallel over batch (8 batch elements -> 8 NeuronCores),
no collectives. Each core runs the full model on its batch element.

Per-core plan (C=64, L=1024, H=8 heads, 4 blocks):
  - Every DynamicDepthSeparableConv1d (pointwise 1x1 + gated depthwise
    k=3/k=15) is folded on the host into a single dense conv
    W2[o,c,t] = pw[o,c]*(g0*w3 + g1*w15)[o,t], evaluated on the PE as a
    K=cin*16 accumulated matmul. The im2col uses a single duplicated
    copy of x with a one-column shift between partition halves so every
    tap-pair K-tile is just an AP offset (no materialized im2col).
  - Attention: dot[lk,lq] in PSUM via row-packed (tile_position) K=64
    matmuls (2 heads concurrently), exp on ScalarE (no max subtraction,
    constant bias -10 instead; softmax normalization is deferred), then
    out = [V^T | 1] @ E with an augmented-ones column providing the
    softmax denominators for free. Division by the denominator uses a
    DVE fast-reciprocal + K=1 broadcast matmul.
  - All matmuls run in float32r (TF32-like, full PE rate at N>=512).
  - ScalarE keeps the natural_log_exp_and_others table set resident:
    rsqrt (instance norm) = exp(-0.5*ln(var+eps)), final sigmoid =
    exp(-ln(1+exp(-z))). No table switches.
"""

import sys

sys.path.insert(0, "/opt/trn_rl_repo")

import contextlib

import numpy as np

import concourse.bass as bass
import concourse.tile as tile
from concourse import mybir
from concourse.bass_utils import run_bass_kernel_spmd

F32 = mybir.dt.float32
F32R = mybir.dt.float32r
AF = mybir.ActivationFunctionType
OP = mybir.AluOpType

B, CIN, C, L = 8, 6, 64, 1024
H, DEPTH = 8, 4
NTAP = 16  # 15 real taps padded to 16 (tap 15 zero weight)
EPS = 1e-5
EXPB = -10.0  # constant exp bias (cancels in softmax division)


# ----------------------------------------------------------------- host prep
def _np(a):
    return np.asarray(a, dtype=np.float32)


def _fold_dds(p, scale=1.0):
    """Fold pointwise+gated depthwise DDS into W2[o, c, 16] (fp32)."""
    pw = _np(p["pw"])  # [cout, cin]
    w3 = _np(p["dw"][0])[:, 0, :]  # [cout, 3]
    w15 = _np(p["dw"][1])[:, 0, :]  # [cout, 15]
    gate = _np(p["gate"])
    g = np.exp(gate - gate.max())
    g = g / g.sum()
    cout = pw.shape[0]
    wc = np.zeros((cout, NTAP), np.float32)
    wc[:, :15] = g[1] * w15
    wc[:, 6:9] += g[0] * w3
    W2 = (pw[:, :, None] * wc[:, None, :]) * scale  # [cout, cin, 16]
    return np.ascontiguousarray(W2)


def _qkv_blob(W2):
    """[cout, 64, 16] -> [128, 8*cout] tap-pair K-tile layout.

    blob[p, kk*cout + o] = W2[o, p%64, 2*kk + (p>=64)]
    """
    cout = W2.shape[0]
    out = np.zeros((128, 8 * cout), np.float32)
    for kk in range(8):
        for half in range(2):
            out[half * 64 : half * 64 + 64, kk * cout : (kk + 1) * cout] = W2[
                :, :, 2 * kk + half
            ].T
    return out


def prep_params(params):
    d = {}
    W2e = _fold_dds(params["enc"])  # [64, 6, 16]
    enc = np.zeros((96, 64), np.float32)
    for t in range(15):
        for c in range(CIN):
            enc[t * CIN + c, :] = W2e[:, c, t]
    d["wenc"] = enc
    s2 = 1.0 / np.sqrt(C)  # both c**-0.25 factors folded into Wq
    for b, bp in enumerate(params["blocks"]):
        a = bp["attn"]
        d[f"wq{b}"] = _qkv_blob(_fold_dds(a["q"], scale=s2))
        d[f"wk{b}"] = _qkv_blob(_fold_dds(a["k"]))
        d[f"wv{b}"] = _qkv_blob(_fold_dds(a["v"]))
        UT = _np(a["u"]).T  # [512, 64]
        d[f"wu{b}"] = np.ascontiguousarray(
            UT.reshape(4, 128, 64).transpose(1, 0, 2).reshape(128, 256)
        )
        d[f"w1_{b}"] = np.ascontiguousarray(_np(bp["w1"]).T)  # [64, 256]
        w2T = _np(bp["w2"]).T  # [256, 64]
        d[f"w2_{b}"] = np.ascontiguousarray(
            w2T.reshape(2, 128, 64).transpose(1, 0, 2).reshape(128, 128)
        )
        d[f"np{b}"] = np.ascontiguousarray(
            np.stack([_np(bp["g1"]), _np(bp["b1"]), _np(bp["g2"]), _np(bp["b2"])], 1)
        )  # [64, 4]
    d["wo"] = np.ascontiguousarray(_np(params["wo"]).T)  # [64, 1]
    d["negbo"] = -_np(params["bo"]).reshape(1, 1)
    d["ones"] = np.ones((1, 64), np.float32)
    d["zx2"] = np.zeros((128, 1040), np.float32)
    d["zx2e"] = np.zeros((96, 1040), np.float32)
    d["vtones"] = np.ones((128, 8 * 520), np.float32)
    return d


# ------------------------------------------------------------ wait splitting
def split_multi_waits(nc):
    """walrus codegen allows one sync wait per instruction; TileContext can
    emit several. Splice single-wait NOPs (same engine) before offenders."""
    ctr = 0
    for func in nc.m.functions:
        for block in func.blocks:
            out = []
            changed = False
            for inst in block.instructions:
                si = inst.sync_info
                if si is not None and si.on_wait is not None and len(si.on_wait) > 1:
                    waits = list(si.on_wait)
                    for w in waits[:-1]:
                        ctr += 1
                        out.append(
                            mybir.InstNoOp(
                                name=f"wsplit-{ctr}",
                                engine=inst.engine,
                                ins=[],
                                outs=[],
                                sync_info=mybir.SyncInfo(on_wait=[w], on_update=[]),
                            )
                        )
                    si.on_wait = [waits[-1]]
                    changed = True
                out.append(inst)
            if changed:
                block.instructions = out
    return ctr


# ------------------------------------------------------------------- kernel
def _instance_norm(nc, work, pre, gamma, beta, outs, epst):
    """pre [64, 1024] fp32 -> inorm with affine -> write each AP in outs."""
    stats = work.tile([64, 2, 6], F32, tag="stats")
    mv = work.tile([64, 2], F32, tag="mv")
    for sg in range(2):
        nc.vector.bn_stats(stats[:, sg, :], pre[:, sg * 512 : (sg + 1) * 512])
    nc.vector.bn_aggr(mv[:], stats[:])
    lnv = work.tile([64, 1], F32, tag="lnv")
    nc.scalar.activation(lnv[:], mv[:, 1:2], AF.Ln, bias=epst[:])
    rstd = work.tile([64, 1], F32, tag="rstd")
    nc.scalar.activation(rstd[:], lnv[:], AF.Exp, scale=-0.5)
    a = work.tile([64, 1], F32, tag="a")
    nc.vector.tensor_mul(a[:], rstd[:], gamma)
    bsh = work.tile([64, 1], F32, tag="bsh")
    nc.vector.tensor_mul(bsh[:], mv[:, 0:1], a[:])
    nc.vector.tensor_sub(bsh[:], beta, bsh[:])
    for n_, o in enumerate(outs):
        eng = nc.vector if n_ == 0 else nc.gpsimd
        eng.tensor_scalar(o, pre[:], a[:], bsh[:], op0=OP.mult, op1=OP.add)


import os
SKIP = set(os.environ.get("KSKIP", "").split(","))


def build_nc():
    nc = bass.Bass()

    x_in = nc.declare_dram_parameter("x", [CIN, L], F32R, False)
    wenc_d = nc.declare_dram_parameter("wenc", [96, 64], F32R, False)
    wq_d, wk_d, wv_d, wu_d, w1_d, w2_d, np_d = [], [], [], [], [], [], []
    for b in range(DEPTH):
        wq_d.append(nc.declare_dram_parameter(f"wq{b}", [128, 4096], F32R, False))
        wk_d.append(nc.declare_dram_parameter(f"wk{b}", [128, 4096], F32R, False))
        wv_d.append(nc.declare_dram_parameter(f"wv{b}", [128, 4096], F32R, False))
        wu_d.append(nc.declare_dram_parameter(f"wu{b}", [128, 256], F32R, False))
        w1_d.append(nc.declare_dram_parameter(f"w1_{b}", [64, 256], F32R, False))
        w2_d.append(nc.declare_dram_parameter(f"w2_{b}", [128, 128], F32R, False))
        np_d.append(nc.declare_dram_parameter(f"np{b}", [64, 4], F32, False))
    wo_d = nc.declare_dram_parameter("wo", [64, 1], F32R, False)
    negbo_d = nc.declare_dram_parameter("negbo", [1, 1], F32, False)
    ones_d = nc.declare_dram_parameter("ones", [1, 64], F32R, False)
    zx2_d = nc.declare_dram_parameter("zx2", [128, 1040], F32R, False)
    zx2e_d = nc.declare_dram_parameter("zx2e", [96, 1040], F32R, False)
    vtones_d = nc.declare_dram_parameter("vtones", [128, 8 * 520], F32R, False)
    out_d = nc.declare_dram_parameter("out", [1, L], F32, True)

    with tile.TileContext(nc) as tc:
        ctx = contextlib.ExitStack()
        const = ctx.enter_context(tc.tile_pool(name="const", bufs=1))
        wpool = ctx.enter_context(tc.tile_pool(name="wpool", bufs=int(os.environ.get("KWBUF", "3"))))
        qk = ctx.enter_context(tc.tile_pool(name="qk", bufs=3))
        epool = ctx.enter_context(tc.tile_pool(name="epool", bufs=int(os.environ.get("KEBUF", "3"))))
        work = ctx.enter_context(tc.tile_pool(name="work", bufs=1))
        _psd_bufs = int(os.environ.get("KPSD", "3"))
        psd = ctx.enter_context(tc.tile_pool(name="psd", bufs=_psd_bufs, space="PSUM"))
        psa = ctx.enter_context(tc.tile_pool(name="psa", bufs=4 - _psd_bufs, space="PSUM"))
        psb = psd

        # ---- constants / persistent state
        X2 = const.tile([128, 1040], F32R)
        X2e = const.tile([96, 1040], F32R)
        vt = const.tile([128, 8 * 520], F32R)
        lns = const.tile([1, 1024], F32)
        rr = const.tile([1, 1024], F32R)
        nc.sync.dma_start(X2[:], zx2_d[:])
        nc.sync.dma_start(X2e[:], zx2e_d[:])
        nc.sync.dma_start(vt[:], vtones_d[:])  # ones-cols at h*65+64 stay 1.0
        ones2 = const.tile([1, 64], F32R)
        nc.sync.dma_start(ones2[:], ones_d[:])
        expb = const.tile([128, 1], F32)
        nc.vector.memset(expb[:], EXPB)
        epst = const.tile([64, 1], F32)
        nc.vector.memset(epst[:], EPS)

        for t in range(15):
            nc.sync.dma_start(
                X2e[t * CIN : (t + 1) * CIN, 15 - t : 15 - t + L], x_in[:, :]
            )
        wenc = const.tile([96, 64], F32R)
        nc.sync.dma_start(wenc[:], wenc_d[:])

        wu_t, w1_t, w2_t, np_t = [], [], [], []
        for b in range(DEPTH):
            wu_t.append(const.tile([128, 256], F32R, name=f"wu{b}"))
            nc.sync.dma_start(wu_t[b][:], wu_d[b][:])
            w1_t.append(const.tile([64, 256], F32R, name=f"w1{b}"))
            nc.sync.dma_start(w1_t[b][:], w1_d[b][:])
            w2_t.append(const.tile([128, 128], F32R, name=f"w2{b}"))
            nc.sync.dma_start(w2_t[b][:], w2_d[b][:])
            np_t.append(const.tile([64, 4], F32, name=f"np{b}"))
            nc.sync.dma_start(np_t[b][:], np_d[b][:])
        wo_t = const.tile([64, 1], F32R)
        nc.sync.dma_start(wo_t[:], wo_d[:])
        negbo = const.tile([1, 1], F32)
        nc.sync.dma_start(negbo[:], negbo_d[:])

        # ---- forward body (optionally repeated for timing)
        import contextlib as _ctxlib
        _rep = int(os.environ.get("KREP", "1"))
        _loop = tc.For_i(0, _rep, 1) if _rep > 1 else _ctxlib.nullcontext()
        with _loop:
            _forward_body(
                nc, tc, const, wpool, qk, epool, work, psd, psa, psb,
                X2, X2e, vt, lns, rr, ones2, expb, epst, wenc,
                wu_t, w1_t, w2_t, np_t, wo_t, negbo,
                wq_d, wk_d, wv_d, out_d,
            )

        ctx.close()

    return nc


def _forward_body(
    nc, tc, const, wpool, qk, epool, work, psd, psa, psb,
    X2, X2e, vt, lns, rr, ones2, expb, epst, wenc,
    wu_t, w1_t, w2_t, np_t, wo_t, negbo,
    wq_d, wk_d, wv_d, out_d,
):
        # ---- encoder: x0 = DDS_enc(x) -> both X2 halves
        for jc in range(2):
            eps_ = psb.tile([64, 512], F32, tag="dot")
            nc.tensor.matmul(
                eps_[:],
                wenc[:],
                X2e[:, 8 + jc * 512 : 8 + (jc + 1) * 512],
                start=True,
                stop=True,
            )
            nc.vector.tensor_copy(X2[0:64, 8 + jc * 512 : 8 + (jc + 1) * 512], eps_[:])
            nc.gpsimd.tensor_copy(
                X2[64:128, 7 + jc * 512 : 7 + (jc + 1) * 512],
                X2[0:64, 8 + jc * 512 : 8 + (jc + 1) * 512],
            )

        # ---- transformer blocks
        for b in range(DEPTH):
            wq = wpool.tile([128, 4096], F32R, tag="w")
            nc.sync.dma_start(wq[:], wq_d[b][:])
            wk = wpool.tile([128, 4096], F32R, tag="w")
            nc.sync.dma_start(wk[:], wk_d[b][:])
            wv = wpool.tile([128, 4096], F32R, tag="w")
            nc.sync.dma_start(wv[:], wv_d[b][:])

            def emit_qk(pair):
                qp = qk.tile([128, 1024], F32R, tag="q", name=f"qp{pair}")
                kp = qk.tile([128, 1024], F32R, tag="k", name=f"kp{pair}")
                for jc in range(2):
                    for wt, dst in ((wq, qp), (wk, kp)):
                        qps = psb.tile([128, 512], F32, tag="dot", name="qps")
                        for kk in range(1 if "qkv" in SKIP else 8):
                            nc.tensor.matmul(
                                qps[:],
                                wt[:, kk * 512 + pair * 128 : kk * 512 + (pair + 1) * 128],
                                X2[:, jc * 512 + 2 * kk + 1 : jc * 512 + 2 * kk + 513],
                                start=(kk == 0),
                                stop=(kk == 7),
                            )
                        nc.vector.tensor_copy(dst[:, jc * 512 : (jc + 1) * 512], qps[:])
                return qp, kp

            qkp = emit_qk(0)

            # v^T via transposed fold: lhsT = X2 slice, rhs = wv K-tile
            for i in range(8):
                vps = psb.tile([128, 512], F32, tag="dot", name="vps")
                for kk in range(1 if "qkv" in SKIP else 8):
                    nc.tensor.matmul(
                        vps[:],
                        X2[:, i * 128 + 2 * kk + 1 : i * 128 + 2 * kk + 1 + 128],
                        wv[:, kk * 512 : (kk + 1) * 512],
                        start=(kk == 0),
                        stop=True,
                    )
                nc.vector.tensor_copy(
                    vt[:, i * 520 : i * 520 + 520]
                    .rearrange("p (h c) -> p h c", h=8)[:, :, 0:64],
                    vps[:].rearrange("p (h c) -> p h c", h=8),
                )

            ui = const.tile([128, 4096], F32R, tag="uin")
            for pair in range(4):
                qp, kp = qkp

                h0, h1 = 2 * pair, 2 * pair + 1
                if "attn" in SKIP:
                    nc.vector.tensor_copy(
                        ui[:, pair * 1024 : (pair + 1) * 1024], qp[:]
                    )
                    if pair < 3:
                        qkp = emit_qk(pair + 1)
                    continue
                for jc in range(2):
                    o12 = psa.tile([65, 1024], F32, tag="o12")
                    for i in range(8):
                        dps = psd.tile([128, 1024], F32, tag="dot")
                        nc.tensor.matmul(
                            dps[:, 0:512],
                            kp[0:64, i * 128 : (i + 1) * 128],
                            qp[0:64, jc * 512 : (jc + 1) * 512],
                            start=True,
                            stop=True,
                            tile_position=(0, 0),
                        )
                        nc.tensor.matmul(
                            dps[:, 512:1024],
                            kp[64:128, i * 128 : (i + 1) * 128],
                            qp[64:128, jc * 512 : (jc + 1) * 512],
                            start=True,
                            stop=True,
                            tile_position=(64, 0),
                        )
                        E = epool.tile([128, 1024], F32R, tag="E")
                        if "exp" in SKIP:
                            nc.vector.tensor_copy(E[:], dps[:])
                        else:
                            nc.scalar.activation(E[:], dps[:], AF.Exp, bias=expb[:])
                        nc.tensor.matmul(
                            o12[:, 0:512],
                            vt[:, i * 520 + h0 * 65 : i * 520 + h0 * 65 + 65],
                            E[:, 0:512],
                            start=(i == 0),
                            stop=(i == 7),
                        )
                        nc.tensor.matmul(
                            o12[:, 512:1024],
                            vt[:, i * 520 + h1 * 65 : i * 520 + h1 * 65 + 65],
                            E[:, 512:1024],
                            start=(i == 0),
                            stop=(i == 7),
                        )
                    nc.scalar.activation(lns[:], o12[64:65, :], AF.Ln)
                    nc.scalar.activation(rr[:], lns[:], AF.Exp, scale=-1.0)
                    f1 = psb.tile([64, 512], F32, tag="dot", name="f1")
                    nc.tensor.matmul(
                        f1[:],
                        ones2[:],
                        rr[:, 0:512],
                        start=True,
                        stop=True,
                    )
                    f2 = psb.tile([64, 512], F32, tag="dot", name="f2")
                    nc.tensor.matmul(
                        f2[:],
                        ones2[:],
                        rr[:, 512:1024],
                        start=True,
                        stop=True,
                    )
                    col = pair * 1024 + jc * 512
                    fs1 = qk.tile([64, 512], F32, tag="fs1")
                    nc.vector.tensor_copy(fs1[:], f1[:])
                    nc.vector.tensor_mul(
                        ui[0:64, col : col + 512], o12[0:64, 0:512], fs1[:]
                    )
                    fs2 = qk.tile([64, 512], F32, tag="fs2")
                    nc.vector.tensor_copy(fs2[:], f2[:])
                    nc.vector.tensor_mul(
                        ui[64:128, col : col + 512], o12[0:64, 512:1024], fs2[:]
                    )
                if pair < 3:
                    qkp = emit_qk(pair + 1)

            # unify + residual + instance-norm 1 -> xn
            pre1 = work.tile([64, 1024], F32, tag="pre1")
            for jc in range(2):
                ups = psb.tile([64, 512], F32, tag="dot")
                for pair in range(4):
                    nc.tensor.matmul(
                        ups[:],
                        wu_t[b][:, pair * 64 : (pair + 1) * 64],
                        ui[:, pair * 1024 + jc * 512 : pair * 1024 + (jc + 1) * 512],
                        start=(pair == 0),
                        stop=(pair == 3),
                    )
                nc.vector.tensor_add(
                    pre1[:, jc * 512 : (jc + 1) * 512],
                    ups[:],
                    X2[0:64, 8 + jc * 512 : 8 + (jc + 1) * 512].bitcast(F32),
                )
            xn = work.tile([64, 1024], F32R, tag="xn")
            npt = np_t[b]
            _instance_norm(nc, work, pre1, npt[:, 0:1], npt[:, 1:2], [xn[:]], epst)

            # ffn
            hdn = work.tile([128, 2048], F32R, tag="hdn")
            for mt in range(2):
                for jc in range(2):
                    hps = psb.tile([128, 512], F32, tag="dot")
                    nc.tensor.matmul(
                        hps[:],
                        w1_t[b][:, mt * 128 : (mt + 1) * 128],
                        xn[:, jc * 512 : (jc + 1) * 512],
                        start=True,
                        stop=True,
                    )
                    nc.vector.tensor_relu(
                        hdn[:, mt * 1024 + jc * 512 : mt * 1024 + (jc + 1) * 512],
                        hps[:],
                    )
            pre2 = work.tile([64, 1024], F32, tag="pre2")
            for jc in range(2):
                fps = psb.tile([64, 512], F32, tag="dot")
                for kk in range(2):
                    nc.tensor.matmul(
                        fps[:],
                        w2_t[b][:, kk * 64 : (kk + 1) * 64],
                        hdn[:, kk * 1024 + jc * 512 : kk * 1024 + (jc + 1) * 512],
                        start=(kk == 0),
                        stop=(kk == 1),
                    )
                nc.vector.tensor_add(
                    pre2[:, jc * 512 : (jc + 1) * 512],
                    fps[:],
                    xn[:, jc * 512 : (jc + 1) * 512].bitcast(F32),
                )
            _instance_norm(
                nc,
                work,
                pre2,
                npt[:, 2:3],
                npt[:, 3:4],
                [X2[0:64, 8 : 8 + 1024], X2[64:128, 7 : 7 + 1024]],
                epst,
            )

        # ---- head: sigmoid(wo @ x + bo) = exp(-ln(1 + exp(-z - bo)))
        usb = work.tile([1, 1024], F32, tag="usb")
        for jc in range(2):
            lg = psb.tile([1, 512], F32, tag="dot")
            nc.tensor.matmul(
                lg[:],
                wo_t[:],
                X2[0:64, 8 + jc * 512 : 8 + (jc + 1) * 512],
                start=True,
                stop=True,
            )
            nc.scalar.activation(
                usb[:, jc * 512 : (jc + 1) * 512],
                lg[:],
                AF.Exp,
                bias=negbo[:],
                scale=-1.0,
            )
        v1 = work.tile([1, 1024], F32, tag="v1")
        nc.vector.tensor_single_scalar(v1[:], usb[:], 1.0, op=OP.add)
        w_ = work.tile([1, 1024], F32, tag="w_")
        nc.scalar.activation(w_[:], v1[:], AF.Ln)
        res = work.tile([1, 1024], F32, tag="res")
        nc.scalar.activation(res[:], w_[:], AF.Exp, scale=-1.0)
        nc.sync.dma_start(out_d[:], res[:])


_CACHE = {}


def kernel(x, params):
    x = np.asarray(x, dtype=np.float32)
    prepped = prep_params(params)
    if "nc" not in _CACHE:
        nc = build_nc()
        split_multi_waits(nc)
        _CACHE["nc"] = nc
    nc = _CACHE["nc"]
    in_maps = [{"x": np.ascontiguousarray(x[i]), **prepped} for i in range(B)]
    res = run_bass_kernel_spmd(nc, in_maps, list(range(B)))
    out = np.stack([r["out"] for r in res.results], axis=0)
    return out.astype(np.float32)


# revision 23
# speedup vs baseline: 1.6946x; 1.6946x over previous
"""DDSCTransformer Trainium2 kernel.

Sharding: data-parallel over batch (8 batch elements -> 8 NeuronCores),
no collectives. Each core runs the full model on its batch element.

Per-core plan (C=64, L=1024, H=8 heads, 4 blocks):
  - Every DynamicDepthSeparableConv1d (pointwise 1x1 + gated depthwise
    k=3/k=15) is folded on the host into a single dense conv
    W2[o,c,t] = pw[o,c]*(g0*w3 + g1*w15)[o,t], evaluated on the PE as a
    K=cin*16 accumulated matmul. The im2col uses a single duplicated
    copy of x with a one-column shift between partition halves so every
    tap-pair K-tile is just an AP offset (no materialized im2col).
  - Attention: dot[lk,lq] in PSUM via row-packed (tile_position) K=64
    matmuls (2 heads concurrently), exp on ScalarE (no max subtraction,
    constant bias -10 instead; softmax normalization is deferred), then
    out = [V^T | 1] @ E with an augmented-ones column providing the
    softmax denominators for free. Division by the denominator uses a
    DVE fast-reciprocal + K=1 broadcast matmul.
  - All matmuls run in float32r (TF32-like, full PE rate at N>=512).
  - ScalarE keeps the natural_log_exp_and_others table set resident:
    rsqrt (instance norm) = exp(-0.5*ln(var+eps)), final sigmoid =
    exp(-ln(1+exp(-z))). No table switches.
"""

import sys

sys.path.insert(0, "/opt/trn_rl_repo")

import contextlib

import numpy as np

import concourse.bass as bass
import concourse.tile as tile
from concourse import mybir
from concourse.bass_utils import run_bass_kernel_spmd

F32 = mybir.dt.float32
F32R = mybir.dt.float32r
AF = mybir.ActivationFunctionType
OP = mybir.AluOpType

B, CIN, C, L = 8, 6, 64, 1024
H, DEPTH = 8, 4
NTAP = 16  # 15 real taps padded to 16 (tap 15 zero weight)
EPS = 1e-5
EXPB = -10.0  # constant exp bias (cancels in softmax division)


# ----------------------------------------------------------------- host prep
def _np(a):
    return np.asarray(a, dtype=np.float32)


def _fold_dds(p, scale=1.0):
    """Fold pointwise+gated depthwise DDS into W2[o, c, 16] (fp32)."""
    pw = _np(p["pw"])  # [cout, cin]
    w3 = _np(p["dw"][0])[:, 0, :]  # [cout, 3]
    w15 = _np(p["dw"][1])[:, 0, :]  # [cout, 15]
    gate = _np(p["gate"])
    g = np.exp(gate - gate.max())
    g = g / g.sum()
    cout = pw.shape[0]
    wc = np.zeros((cout, NTAP), np.float32)
    wc[:, :15] = g[1] * w15
    wc[:, 6:9] += g[0] * w3
    W2 = (pw[:, :, None] * wc[:, None, :]) * scale  # [cout, cin, 16]
    return np.ascontiguousarray(W2)


def _qkv_blob(W2):
    """[cout, 64, 16] -> [128, 8*cout] tap-pair K-tile layout.

    blob[p, kk*cout + o] = W2[o, p%64, 2*kk + (p>=64)]
    """
    cout = W2.shape[0]
    out = np.zeros((128, 8 * cout), np.float32)
    for kk in range(8):
        for half in range(2):
            out[half * 64 : half * 64 + 64, kk * cout : (kk + 1) * cout] = W2[
                :, :, 2 * kk + half
            ].T
    return out


def prep_params(params):
    d = {}
    W2e = _fold_dds(params["enc"])  # [64, 6, 16]
    enc = np.zeros((96, 64), np.float32)
    for t in range(15):
        for c in range(CIN):
            enc[t * CIN + c, :] = W2e[:, c, t]
    d["wenc"] = enc
    s2 = 1.0 / np.sqrt(C)  # both c**-0.25 factors folded into Wq
    for b, bp in enumerate(params["blocks"]):
        a = bp["attn"]
        d[f"wq{b}"] = _qkv_blob(_fold_dds(a["q"], scale=s2))
        d[f"wk{b}"] = _qkv_blob(_fold_dds(a["k"]))
        d[f"wv{b}"] = _qkv_blob(_fold_dds(a["v"]))
        UT = _np(a["u"]).T  # [512, 64]
        d[f"wu{b}"] = np.ascontiguousarray(
            UT.reshape(4, 128, 64).transpose(1, 0, 2).reshape(128, 256)
        )
        d[f"w1_{b}"] = np.ascontiguousarray(_np(bp["w1"]).T)  # [64, 256]
        w2T = _np(bp["w2"]).T  # [256, 64]
        d[f"w2_{b}"] = np.ascontiguousarray(
            w2T.reshape(2, 128, 64).transpose(1, 0, 2).reshape(128, 128)
        )
        d[f"np{b}"] = np.ascontiguousarray(
            np.stack([_np(bp["g1"]), _np(bp["b1"]), _np(bp["g2"]), _np(bp["b2"])], 1)
        )  # [64, 4]
    d["wo"] = np.ascontiguousarray(_np(params["wo"]).T)  # [64, 1]
    d["negbo"] = -_np(params["bo"]).reshape(1, 1)
    d["ones"] = np.ones((1, 64), np.float32)
    d["zx2"] = np.zeros((128, 1040), np.float32)
    d["zx2e"] = np.zeros((96, 1040), np.float32)
    d["vtones"] = np.ones((128, 8 * 520), np.float32)
    return d


# ------------------------------------------------------------ wait splitting
def split_multi_waits(nc):
    """walrus codegen allows one sync wait per instruction; TileContext can
    emit several. Splice single-wait NOPs (same engine) before offenders."""
    ctr = 0
    for func in nc.m.functions:
        for block in func.blocks:
            out = []
            changed = False
            for inst in block.instructions:
                si = inst.sync_info
                if si is not None and si.on_wait is not None and len(si.on_wait) > 1:
                    waits = list(si.on_wait)
                    for w in waits[:-1]:
                        ctr += 1
                        out.append(
                            mybir.InstNoOp(
                                name=f"wsplit-{ctr}",
                                engine=inst.engine,
                                ins=[],
                                outs=[],
                                sync_info=mybir.SyncInfo(on_wait=[w], on_update=[]),
                            )
                        )
                    si.on_wait = [waits[-1]]
                    changed = True
                out.append(inst)
            if changed:
                block.instructions = out
    return ctr


# ------------------------------------------------------------------- kernel
def _instance_norm(nc, work, pre, gamma, beta, outs, epst):
    """pre [64, 1024] fp32 -> inorm with affine -> write each AP in outs."""
    stats = work.tile([64, 2, 6], F32, tag="stats")
    mv = work.tile([64, 2], F32, tag="mv")
    for sg in range(2):
        nc.vector.bn_stats(stats[:, sg, :], pre[:, sg * 512 : (sg + 1) * 512])
    nc.vector.bn_aggr(mv[:], stats[:])
    lnv = work.tile([64, 1], F32, tag="lnv")
    nc.scalar.activation(lnv[:], mv[:, 1:2], AF.Ln, bias=epst[:])
    rstd = work.tile([64, 1], F32, tag="rstd")
    nc.scalar.activation(rstd[:], lnv[:], AF.Exp, scale=-0.5)
    a = work.tile([64, 1], F32, tag="a")
    nc.vector.tensor_mul(a[:], rstd[:], gamma)
    bsh = work.tile([64, 1], F32, tag="bsh")
    nc.vector.tensor_mul(bsh[:], mv[:, 0:1], a[:])
    nc.vector.tensor_sub(bsh[:], beta, bsh[:])
    for o in outs:
        nc.vector.tensor_scalar(o, pre[:], a[:], bsh[:], op0=OP.mult, op1=OP.add)


import os
SKIP = set(os.environ.get("KSKIP", "").split(","))


def build_nc():
    nc = bass.Bass()

    x_in = nc.declare_dram_parameter("x", [CIN, L], F32R, False)
    wenc_d = nc.declare_dram_parameter("wenc", [96, 64], F32R, False)
    wq_d, wk_d, wv_d, wu_d, w1_d, w2_d, np_d = [], [], [], [], [], [], []
    for b in range(DEPTH):
        wq_d.append(nc.declare_dram_parameter(f"wq{b}", [128, 4096], F32R, False))
        wk_d.append(nc.declare_dram_parameter(f"wk{b}", [128, 4096], F32R, False))
        wv_d.append(nc.declare_dram_parameter(f"wv{b}", [128, 4096], F32R, False))
        wu_d.append(nc.declare_dram_parameter(f"wu{b}", [128, 256], F32R, False))
        w1_d.append(nc.declare_dram_parameter(f"w1_{b}", [64, 256], F32R, False))
        w2_d.append(nc.declare_dram_parameter(f"w2_{b}", [128, 128], F32R, False))
        np_d.append(nc.declare_dram_parameter(f"np{b}", [64, 4], F32, False))
    wo_d = nc.declare_dram_parameter("wo", [64, 1], F32R, False)
    negbo_d = nc.declare_dram_parameter("negbo", [1, 1], F32, False)
    ones_d = nc.declare_dram_parameter("ones", [1, 64], F32R, False)
    zx2_d = nc.declare_dram_parameter("zx2", [128, 1040], F32R, False)
    zx2e_d = nc.declare_dram_parameter("zx2e", [96, 1040], F32R, False)
    vtones_d = nc.declare_dram_parameter("vtones", [128, 8 * 520], F32R, False)
    out_d = nc.declare_dram_parameter("out", [1, L], F32, True)

    with tile.TileContext(nc) as tc:
        ctx = contextlib.ExitStack()
        const = ctx.enter_context(tc.tile_pool(name="const", bufs=1))
        wpool = ctx.enter_context(tc.tile_pool(name="wpool", bufs=int(os.environ.get("KWBUF", "3"))))
        qk = ctx.enter_context(tc.tile_pool(name="qk", bufs=2))
        epool = ctx.enter_context(tc.tile_pool(name="epool", bufs=int(os.environ.get("KEBUF", "3"))))
        work = ctx.enter_context(tc.tile_pool(name="work", bufs=1))
        _psd_bufs = int(os.environ.get("KPSD", "3"))
        psd = ctx.enter_context(tc.tile_pool(name="psd", bufs=_psd_bufs, space="PSUM"))
        psa = ctx.enter_context(tc.tile_pool(name="psa", bufs=4 - _psd_bufs, space="PSUM"))
        psb = psd

        # ---- constants / persistent state
        X2 = const.tile([128, 1040], F32R)
        X2e = const.tile([96, 1040], F32R)
        vt = const.tile([128, 8 * 520], F32R)
        lns = const.tile([1, 1024], F32)
        rr = const.tile([1, 1024], F32R)
        nc.sync.dma_start(X2[:], zx2_d[:])
        nc.sync.dma_start(X2e[:], zx2e_d[:])
        nc.sync.dma_start(vt[:], vtones_d[:])  # ones-cols at h*65+64 stay 1.0
        ones2 = const.tile([1, 64], F32R)
        nc.sync.dma_start(ones2[:], ones_d[:])
        expb = const.tile([128, 1], F32)
        nc.vector.memset(expb[:], EXPB)
        epst = const.tile([64, 1], F32)
        nc.vector.memset(epst[:], EPS)

        for t in range(15):
            nc.sync.dma_start(
                X2e[t * CIN : (t + 1) * CIN, 15 - t : 15 - t + L], x_in[:, :]
            )
        wenc = const.tile([96, 64], F32R)
        nc.sync.dma_start(wenc[:], wenc_d[:])

        wu_t, w1_t, w2_t, np_t = [], [], [], []
        for b in range(DEPTH):
            wu_t.append(const.tile([128, 256], F32R, name=f"wu{b}"))
            nc.sync.dma_start(wu_t[b][:], wu_d[b][:])
            w1_t.append(const.tile([64, 256], F32R, name=f"w1{b}"))
            nc.sync.dma_start(w1_t[b][:], w1_d[b][:])
            w2_t.append(const.tile([128, 128], F32R, name=f"w2{b}"))
            nc.sync.dma_start(w2_t[b][:], w2_d[b][:])
            np_t.append(const.tile([64, 4], F32, name=f"np{b}"))
            nc.sync.dma_start(np_t[b][:], np_d[b][:])
        wo_t = const.tile([64, 1], F32R)
        nc.sync.dma_start(wo_t[:], wo_d[:])
        negbo = const.tile([1, 1], F32)
        nc.sync.dma_start(negbo[:], negbo_d[:])

        # ---- forward body (optionally repeated for timing)
        import contextlib as _ctxlib
        _rep = int(os.environ.get("KREP", "1"))
        _loop = tc.For_i(0, _rep, 1) if _rep > 1 else _ctxlib.nullcontext()
        with _loop:
            _forward_body(
                nc, tc, const, wpool, qk, epool, work, psd, psa, psb,
                X2, X2e, vt, lns, rr, ones2, expb, epst, wenc,
                wu_t, w1_t, w2_t, np_t, wo_t, negbo,
                wq_d, wk_d, wv_d, out_d,
            )

        ctx.close()

    return nc


def _forward_body(
    nc, tc, const, wpool, qk, epool, work, psd, psa, psb,
    X2, X2e, vt, lns, rr, ones2, expb, epst, wenc,
    wu_t, w1_t, w2_t, np_t, wo_t, negbo,
    wq_d, wk_d, wv_d, out_d,
):
        # ---- encoder: x0 = DDS_enc(x) -> both X2 halves
        for jc in range(2):
            eps_ = psb.tile([64, 512], F32, tag="dot")
            nc.tensor.matmul(
                eps_[:],
                wenc[:],
                X2e[:, 8 + jc * 512 : 8 + (jc + 1) * 512],
                start=True,
                stop=True,
            )
            nc.vector.tensor_copy(X2[0:64, 8 + jc * 512 : 8 + (jc + 1) * 512], eps_[:])
            nc.vector.tensor_copy(
                X2[64:128, 7 + jc * 512 : 7 + (jc + 1) * 512], eps_[:]
            )

        # ---- transformer blocks
        for b in range(DEPTH):
            wq = wpool.tile([128, 4096], F32R, tag="w")
            nc.sync.dma_start(wq[:], wq_d[b][:])
            wk = wpool.tile([128, 4096], F32R, tag="w")
            nc.sync.dma_start(wk[:], wk_d[b][:])
            wv = wpool.tile([128, 4096], F32R, tag="w")
            nc.sync.dma_start(wv[:], wv_d[b][:])

            def emit_qk(pair):
                qp = qk.tile([128, 1024], F32R, tag="q", name=f"qp{pair}")
                kp = qk.tile([128, 1024], F32R, tag="k", name=f"kp{pair}")
                for jc in range(2):
                    for wt, dst in ((wq, qp), (wk, kp)):
                        qps = psb.tile([128, 512], F32, tag="dot", name="qps")
                        for kk in range(1 if "qkv" in SKIP else 8):
                            nc.tensor.matmul(
                                qps[:],
                                wt[:, kk * 512 + pair * 128 : kk * 512 + (pair + 1) * 128],
                                X2[:, jc * 512 + 2 * kk + 1 : jc * 512 + 2 * kk + 513],
                                start=(kk == 0),
                                stop=(kk == 7),
                            )
                        nc.vector.tensor_copy(dst[:, jc * 512 : (jc + 1) * 512], qps[:])
                return qp, kp

            qkp = emit_qk(0)

            # v^T via transposed fold: lhsT = X2 slice, rhs = wv K-tile
            for i in range(8):
                vps = psb.tile([128, 512], F32, tag="dot", name="vps")
                for kk in range(1 if "qkv" in SKIP else 8):
                    nc.tensor.matmul(
                        vps[:],
                        X2[:, i * 128 + 2 * kk + 1 : i * 128 + 2 * kk + 1 + 128],
                        wv[:, kk * 512 : (kk + 1) * 512],
                        start=(kk == 0),
                        stop=True,
                    )
                nc.vector.tensor_copy(
                    vt[:, i * 520 : i * 520 + 520]
                    .rearrange("p (h c) -> p h c", h=8)[:, :, 0:64],
                    vps[:].rearrange("p (h c) -> p h c", h=8),
                )

            ui = const.tile([128, 4096], F32R, tag="uin")
            for pair in range(4):
                qp, kp = qkp

                h0, h1 = 2 * pair, 2 * pair + 1
                if "attn" in SKIP:
                    nc.vector.tensor_copy(
                        ui[:, pair * 1024 : (pair + 1) * 1024], qp[:]
                    )
                    if pair < 3:
                        qkp = emit_qk(pair + 1)
                    continue
                for jc in range(2):
                    o12 = psa.tile([65, 1024], F32, tag="o12")
                    for i in range(8):
                        dps = psd.tile([128, 1024], F32, tag="dot")
                        nc.tensor.matmul(
                            dps[:, 0:512],
                            kp[0:64, i * 128 : (i + 1) * 128],
                            qp[0:64, jc * 512 : (jc + 1) * 512],
                            start=True,
                            stop=True,
                            tile_position=(0, 0),
                        )
                        nc.tensor.matmul(
                            dps[:, 512:1024],
                            kp[64:128, i * 128 : (i + 1) * 128],
                            qp[64:128, jc * 512 : (jc + 1) * 512],
                            start=True,
                            stop=True,
                            tile_position=(64, 0),
                        )
                        E = epool.tile([128, 1024], F32R, tag="E")
                        if "exp" in SKIP:
                            nc.vector.tensor_copy(E[:], dps[:])
                        else:
                            nc.scalar.activation(E[:], dps[:], AF.Exp, bias=expb[:])
                        nc.tensor.matmul(
                            o12[:, 0:512],
                            vt[:, i * 520 + h0 * 65 : i * 520 + h0 * 65 + 65],
                            E[:, 0:512],
                            start=(i == 0),
                            stop=(i == 7),
                        )
                        nc.tensor.matmul(
                            o12[:, 512:1024],
                            vt[:, i * 520 + h1 * 65 : i * 520 + h1 * 65 + 65],
                            E[:, 512:1024],
                            start=(i == 0),
                            stop=(i == 7),
                        )
                    nc.scalar.activation(lns[:], o12[64:65, :], AF.Ln)
                    nc.scalar.activation(rr[:], lns[:], AF.Exp, scale=-1.0)
                    f1 = psb.tile([64, 512], F32, tag="dot", name="f1")
                    nc.tensor.matmul(
                        f1[:],
                        ones2[:],
                        rr[:, 0:512],
                        start=True,
                        stop=True,
                    )
                    f2 = psb.tile([64, 512], F32, tag="dot", name="f2")
                    nc.tensor.matmul(
                        f2[:],
                        ones2[:],
                        rr[:, 512:1024],
                        start=True,
                        stop=True,
                    )
                    col = pair * 1024 + jc * 512
                    fs1 = qk.tile([64, 512], F32, tag="fs1")
                    nc.vector.tensor_copy(fs1[:], f1[:])
                    nc.vector.tensor_mul(
                        ui[0:64, col : col + 512], o12[0:64, 0:512], fs1[:]
                    )
                    fs2 = qk.tile([64, 512], F32, tag="fs2")
                    nc.vector.tensor_copy(fs2[:], f2[:])
                    nc.vector.tensor_mul(
                        ui[64:128, col : col + 512], o12[0:64, 512:1024], fs2[:]
                    )
                if pair < 3:
                    qkp = emit_qk(pair + 1)

            # unify + residual + instance-norm 1 -> xn
            pre1 = work.tile([64, 1024], F32, tag="pre1")
            for jc in range(2):
                ups = psb.tile([64, 512], F32, tag="dot")
                for pair in range(4):
                    nc.tensor.matmul(
                        ups[:],
                        wu_t[b][:, pair * 64 : (pair + 1) * 64],
                        ui[:, pair * 1024 + jc * 512 : pair * 1024 + (jc + 1) * 512],
                        start=(pair == 0),
                        stop=(pair == 3),
                    )
                nc.vector.tensor_add(
                    pre1[:, jc * 512 : (jc + 1) * 512],
                    ups[:],
                    X2[0:64, 8 + jc * 512 : 8 + (jc + 1) * 512].bitcast(F32),
                )
            xn = work.tile([64, 1024], F32R, tag="xn")
            npt = np_t[b]
            _instance_norm(nc, work, pre1, npt[:, 0:1], npt[:, 1:2], [xn[:]], epst)

            # ffn
            hdn = work.tile([128, 2048], F32R, tag="hdn")
            for mt in range(2):
                for jc in range(2):
                    hps = psb.tile([128, 512], F32, tag="dot")
                    nc.tensor.matmul(
                        hps[:],
                        w1_t[b][:, mt * 128 : (mt + 1) * 128],
                        xn[:, jc * 512 : (jc + 1) * 512],
                        start=True,
                        stop=True,
                    )
                    nc.vector.tensor_relu(
                        hdn[:, mt * 1024 + jc * 512 : mt * 1024 + (jc + 1) * 512],
                        hps[:],
                    )
            pre2 = work.tile([64, 1024], F32, tag="pre2")
            for jc in range(2):
                fps = psb.tile([64, 512], F32, tag="dot")
                for kk in range(2):
                    nc.tensor.matmul(
                        fps[:],
                        w2_t[b][:, kk * 64 : (kk + 1) * 64],
                        hdn[:, kk * 1024 + jc * 512 : kk * 1024 + (jc + 1) * 512],
                        start=(kk == 0),
                        stop=(kk == 1),
                    )
                nc.vector.tensor_add(
                    pre2[:, jc * 512 : (jc + 1) * 512],
                    fps[:],
                    xn[:, jc * 512 : (jc + 1) * 512].bitcast(F32),
                )
            _instance_norm(
                nc,
                work,
                pre2,
                npt[:, 2:3],
                npt[:, 3:4],
                [X2[0:64, 8 : 8 + 1024], X2[64:128, 7 : 7 + 1024]],
                epst,
            )

        # ---- head: sigmoid(wo @ x + bo) = exp(-ln(1 + exp(-z - bo)))
        usb = work.tile([1, 1024], F32, tag="usb")
        for jc in range(2):
            lg = psb.tile([1, 512], F32, tag="dot")
            nc.tensor.matmul(
                lg[:],
                wo_t[:],
                X2[0:64, 8 + jc * 512 : 8 + (jc + 1) * 512],
                start=True,
                stop=True,
            )
            nc.scalar.activation(
                usb[:, jc * 512 : (jc + 1) * 512],
                lg[:],
                AF.Exp,
                bias=negbo[:],
                scale=-1.0,
            )
        v1 = work.tile([1, 1024], F32, tag="v1")
        nc.vector.tensor_single_scalar(v1[:], usb[:], 1.0, op=OP.add)
        w_ = work.tile([1, 1024], F32, tag="w_")
        nc.scalar.activation(w_[:], v1[:], AF.Ln)
        res = work.tile([1, 1024], F32, tag="res")
        nc.scalar.activation(res[:], w_[:], AF.Exp, scale=-1.0)
        nc.sync.dma_start(out_d[:], res[:])


_CACHE = {}


def kernel(x, params):
    x = np.asarray(x, dtype=np.float32)
    prepped = prep_params(params)
    if "nc" not in _CACHE:
        nc = build_nc()
        split_multi_waits(nc)
        _CACHE["nc"] = nc
    nc = _CACHE["nc"]
    in_maps = [{"x": np.ascontiguousarray(x[i]), **prepped} for i in range(B)]
    res = run_bass_kernel_spmd(nc, in_maps, list(range(B)))
    out = np.stack([r["out"] for r in res.results], axis=0)
    return out.astype(np.float32)


# revision 24
# speedup vs baseline: 423.6607x; 250.0032x over previous
"""DDSCTransformer Trainium2 kernel.

Sharding: data-parallel over batch (8 batch elements -> 8 NeuronCores),
no collectives. Each core runs the full model on its batch element.

Per-core plan (C=64, L=1024, H=8 heads, 4 blocks):
  - Every DynamicDepthSeparableConv1d (pointwise 1x1 + gated depthwise
    k=3/k=15) is folded on the host into a single dense conv
    W2[o,c,t] = pw[o,c]*(g0*w3 + g1*w15)[o,t], evaluated on the PE as a
    K=cin*16 accumulated matmul. The im2col uses a single duplicated
    copy of x with a one-column shift between partition halves so every
    tap-pair K-tile is just an AP offset (no materialized im2col).
  - Attention: dot[lk,lq] in PSUM via row-packed (tile_position) K=64
    matmuls (2 heads concurrently), exp on ScalarE (no max subtraction,
    constant bias -10 instead; softmax normalization is deferred), then
    out = [V^T | 1] @ E with an augmented-ones column providing the
    softmax denominators for free. Division by the denominator uses a
    DVE fast-reciprocal + K=1 broadcast matmul.
  - All matmuls run in float32r (TF32-like, full PE rate at N>=512).
  - ScalarE keeps the natural_log_exp_and_others table set resident:
    rsqrt (instance norm) = exp(-0.5*ln(var+eps)), final sigmoid =
    exp(-ln(1+exp(-z))). No table switches.
"""

import sys

sys.path.insert(0, "/opt/trn_rl_repo")

import contextlib

import numpy as np

import concourse.bass as bass
import concourse.tile as tile
from concourse import mybir
from concourse.bass_utils import run_bass_kernel_spmd

F32 = mybir.dt.float32
F32R = mybir.dt.float32r
AF = mybir.ActivationFunctionType
OP = mybir.AluOpType

B, CIN, C, L = 8, 6, 64, 1024
H, DEPTH = 8, 4
NTAP = 16  # 15 real taps padded to 16 (tap 15 zero weight)
EPS = 1e-5
EXPB = -10.0  # constant exp bias (cancels in softmax division)


# ----------------------------------------------------------------- host prep
def _np(a):
    return np.asarray(a, dtype=np.float32)


def _fold_dds(p, scale=1.0):
    """Fold pointwise+gated depthwise DDS into W2[o, c, 16] (fp32)."""
    pw = _np(p["pw"])  # [cout, cin]
    w3 = _np(p["dw"][0])[:, 0, :]  # [cout, 3]
    w15 = _np(p["dw"][1])[:, 0, :]  # [cout, 15]
    gate = _np(p["gate"])
    g = np.exp(gate - gate.max())
    g = g / g.sum()
    cout = pw.shape[0]
    wc = np.zeros((cout, NTAP), np.float32)
    wc[:, :15] = g[1] * w15
    wc[:, 6:9] += g[0] * w3
    W2 = (pw[:, :, None] * wc[:, None, :]) * scale  # [cout, cin, 16]
    return np.ascontiguousarray(W2)


def _qkv_blob(W2):
    """[cout, 64, 16] -> [128, 8*cout] tap-pair K-tile layout.

    blob[p, kk*cout + o] = W2[o, p%64, 2*kk + (p>=64)]
    """
    cout = W2.shape[0]
    out = np.zeros((128, 8 * cout), np.float32)
    for kk in range(8):
        for half in range(2):
            out[half * 64 : half * 64 + 64, kk * cout : (kk + 1) * cout] = W2[
                :, :, 2 * kk + half
            ].T
    return out


def prep_params(params):
    d = {}
    W2e = _fold_dds(params["enc"])  # [64, 6, 16]
    enc = np.zeros((96, 64), np.float32)
    for t in range(15):
        for c in range(CIN):
            enc[t * CIN + c, :] = W2e[:, c, t]
    d["wenc"] = enc
    s2 = 1.0 / np.sqrt(C)  # both c**-0.25 factors folded into Wq
    for b, bp in enumerate(params["blocks"]):
        a = bp["attn"]
        d[f"wq{b}"] = _qkv_blob(_fold_dds(a["q"], scale=s2))
        d[f"wk{b}"] = _qkv_blob(_fold_dds(a["k"]))
        d[f"wv{b}"] = _qkv_blob(_fold_dds(a["v"]))
        UT = _np(a["u"]).T  # [512, 64]
        d[f"wu{b}"] = np.ascontiguousarray(
            UT.reshape(4, 128, 64).transpose(1, 0, 2).reshape(128, 256)
        )
        d[f"w1_{b}"] = np.ascontiguousarray(_np(bp["w1"]).T)  # [64, 256]
        w2T = _np(bp["w2"]).T  # [256, 64]
        d[f"w2_{b}"] = np.ascontiguousarray(
            w2T.reshape(2, 128, 64).transpose(1, 0, 2).reshape(128, 128)
        )
        d[f"np{b}"] = np.ascontiguousarray(
            np.stack([_np(bp["g1"]), _np(bp["b1"]), _np(bp["g2"]), _np(bp["b2"])], 1)
        )  # [64, 4]
    d["wo"] = np.ascontiguousarray(_np(params["wo"]).T)  # [64, 1]
    d["negbo"] = -_np(params["bo"]).reshape(1, 1)
    d["ones"] = np.ones((1, 64), np.float32)
    d["zx2"] = np.zeros((128, 1040), np.float32)
    d["zx2e"] = np.zeros((96, 1040), np.float32)
    d["vtones"] = np.ones((128, 8 * 520), np.float32)
    return d


# ------------------------------------------------------------ wait splitting
def split_multi_waits(nc):
    """walrus codegen allows one sync wait per instruction; TileContext can
    emit several. Splice single-wait NOPs (same engine) before offenders."""
    ctr = 0
    for func in nc.m.functions:
        for block in func.blocks:
            out = []
            changed = False
            for inst in block.instructions:
                si = inst.sync_info
                if si is not None and si.on_wait is not None and len(si.on_wait) > 1:
                    waits = list(si.on_wait)
                    for w in waits[:-1]:
                        ctr += 1
                        out.append(
                            mybir.InstNoOp(
                                name=f"wsplit-{ctr}",
                                engine=inst.engine,
                                ins=[],
                                outs=[],
                                sync_info=mybir.SyncInfo(on_wait=[w], on_update=[]),
                            )
                        )
                    si.on_wait = [waits[-1]]
                    changed = True
                out.append(inst)
            if changed:
                block.instructions = out
    return ctr


# ------------------------------------------------------------------- kernel
def _instance_norm(nc, work, pre, gamma, beta, outs, epst):
    """pre [64, 1024] fp32 -> inorm with affine -> write each AP in outs."""
    stats = work.tile([64, 2, 6], F32, tag="stats")
    mv = work.tile([64, 2], F32, tag="mv")
    for sg in range(2):
        nc.vector.bn_stats(stats[:, sg, :], pre[:, sg * 512 : (sg + 1) * 512])
    nc.vector.bn_aggr(mv[:], stats[:])
    lnv = work.tile([64, 1], F32, tag="lnv")
    nc.scalar.activation(lnv[:], mv[:, 1:2], AF.Ln, bias=epst[:])
    rstd = work.tile([64, 1], F32, tag="rstd")
    nc.scalar.activation(rstd[:], lnv[:], AF.Exp, scale=-0.5)
    a = work.tile([64, 1], F32, tag="a")
    nc.vector.tensor_mul(a[:], rstd[:], gamma)
    bsh = work.tile([64, 1], F32, tag="bsh")
    nc.vector.tensor_mul(bsh[:], mv[:, 0:1], a[:])
    nc.vector.tensor_sub(bsh[:], beta, bsh[:])
    for o in outs:
        nc.vector.tensor_scalar(o, pre[:], a[:], bsh[:], op0=OP.mult, op1=OP.add)


import os
SKIP = set(os.environ.get("KSKIP", "").split(","))


def build_nc():
    nc = bass.Bass()

    x_in = nc.declare_dram_parameter("x", [CIN, L], F32R, False)
    wenc_d = nc.declare_dram_parameter("wenc", [96, 64], F32R, False)
    wq_d, wk_d, wv_d, wu_d, w1_d, w2_d, np_d = [], [], [], [], [], [], []
    for b in range(DEPTH):
        wq_d.append(nc.declare_dram_parameter(f"wq{b}", [128, 4096], F32R, False))
        wk_d.append(nc.declare_dram_parameter(f"wk{b}", [128, 4096], F32R, False))
        wv_d.append(nc.declare_dram_parameter(f"wv{b}", [128, 4096], F32R, False))
        wu_d.append(nc.declare_dram_parameter(f"wu{b}", [128, 256], F32R, False))
        w1_d.append(nc.declare_dram_parameter(f"w1_{b}", [64, 256], F32R, False))
        w2_d.append(nc.declare_dram_parameter(f"w2_{b}", [128, 128], F32R, False))
        np_d.append(nc.declare_dram_parameter(f"np{b}", [64, 4], F32, False))
    wo_d = nc.declare_dram_parameter("wo", [64, 1], F32R, False)
    negbo_d = nc.declare_dram_parameter("negbo", [1, 1], F32, False)
    ones_d = nc.declare_dram_parameter("ones", [1, 64], F32R, False)
    zx2_d = nc.declare_dram_parameter("zx2", [128, 1040], F32R, False)
    zx2e_d = nc.declare_dram_parameter("zx2e", [96, 1040], F32R, False)
    vtones_d = nc.declare_dram_parameter("vtones", [128, 8 * 520], F32R, False)
    out_d = nc.declare_dram_parameter("out", [1, L], F32, True)

    with tile.TileContext(nc) as tc:
        ctx = contextlib.ExitStack()
        const = ctx.enter_context(tc.tile_pool(name="const", bufs=1))
        wpool = ctx.enter_context(tc.tile_pool(name="wpool", bufs=int(os.environ.get("KWBUF", "3"))))
        qk = ctx.enter_context(tc.tile_pool(name="qk", bufs=2))
        epool = ctx.enter_context(tc.tile_pool(name="epool", bufs=int(os.environ.get("KEBUF", "3"))))
        work = ctx.enter_context(tc.tile_pool(name="work", bufs=1))
        _psd_bufs = int(os.environ.get("KPSD", "3"))
        psd = ctx.enter_context(tc.tile_pool(name="psd", bufs=_psd_bufs, space="PSUM"))
        psa = ctx.enter_context(tc.tile_pool(name="psa", bufs=4 - _psd_bufs, space="PSUM"))
        psb = psd

        # ---- constants / persistent state
        X2 = const.tile([128, 1040], F32R)
        X2e = const.tile([96, 1040], F32R)
        vt = const.tile([128, 8 * 520], F32R)
        lns = const.tile([1, 1024], F32)
        rr = const.tile([1, 1024], F32R)
        nc.sync.dma_start(X2[:], zx2_d[:])
        nc.sync.dma_start(X2e[:], zx2e_d[:])
        nc.sync.dma_start(vt[:], vtones_d[:])  # ones-cols at h*65+64 stay 1.0
        ones2 = const.tile([1, 64], F32R)
        nc.sync.dma_start(ones2[:], ones_d[:])
        expb = const.tile([128, 1], F32)
        nc.vector.memset(expb[:], EXPB)
        epst = const.tile([64, 1], F32)
        nc.vector.memset(epst[:], EPS)

        for t in range(15):
            nc.sync.dma_start(
                X2e[t * CIN : (t + 1) * CIN, 15 - t : 15 - t + L], x_in[:, :]
            )
        wenc = const.tile([96, 64], F32R)
        nc.sync.dma_start(wenc[:], wenc_d[:])

        wu_t, w1_t, w2_t, np_t = [], [], [], []
        for b in range(DEPTH):
            wu_t.append(const.tile([128, 256], F32R, name=f"wu{b}"))
            nc.sync.dma_start(wu_t[b][:], wu_d[b][:])
            w1_t.append(const.tile([64, 256], F32R, name=f"w1{b}"))
            nc.sync.dma_start(w1_t[b][:], w1_d[b][:])
            w2_t.append(const.tile([128, 128], F32R, name=f"w2{b}"))
            nc.sync.dma_start(w2_t[b][:], w2_d[b][:])
            np_t.append(const.tile([64, 4], F32, name=f"np{b}"))
            nc.sync.dma_start(np_t[b][:], np_d[b][:])
        wo_t = const.tile([64, 1], F32R)
        nc.sync.dma_start(wo_t[:], wo_d[:])
        negbo = const.tile([1, 1], F32)
        nc.sync.dma_start(negbo[:], negbo_d[:])

        # ---- forward body (optionally repeated for timing)
        import contextlib as _ctxlib
        _rep = int(os.environ.get("KREP", "1"))
        _loop = tc.For_i(0, _rep, 1) if _rep > 1 else _ctxlib.nullcontext()
        with _loop:
            _forward_body(
                nc, tc, const, wpool, qk, epool, work, psd, psa, psb,
                X2, X2e, vt, lns, rr, ones2, expb, epst, wenc,
                wu_t, w1_t, w2_t, np_t, wo_t, negbo,
                wq_d, wk_d, wv_d, out_d,
            )

        ctx.close()

    return nc


def _forward_body(
    nc, tc, const, wpool, qk, epool, work, psd, psa, psb,
    X2, X2e, vt, lns, rr, ones2, expb, epst, wenc,
    wu_t, w1_t, w2_t, np_t, wo_t, negbo,
    wq_d, wk_d, wv_d, out_d,
):
        # ---- encoder: x0 = DDS_enc(x) -> both X2 halves
        for jc in range(2):
            eps_ = psb.tile([64, 512], F32, tag="dot")
            nc.tensor.matmul(
                eps_[:],
                wenc[:],
                X2e[:, 8 + jc * 512 : 8 + (jc + 1) * 512],
                start=True,
                stop=True,
            )
            nc.vector.tensor_copy(X2[0:64, 8 + jc * 512 : 8 + (jc + 1) * 512], eps_[:])
            nc.vector.tensor_copy(
                X2[64:128, 7 + jc * 512 : 7 + (jc + 1) * 512], eps_[:]
            )

        # ---- transformer blocks
        for b in range(DEPTH):
            wq = wpool.tile([128, 4096], F32R, tag="w")
            nc.sync.dma_start(wq[:], wq_d[b][:])
            wk = wpool.tile([128, 4096], F32R, tag="w")
            nc.sync.dma_start(wk[:], wk_d[b][:])
            wv = wpool.tile([128, 4096], F32R, tag="w")
            nc.sync.dma_start(wv[:], wv_d[b][:])

            def emit_qk(pair):
                qp = qk.tile([128, 1024], F32R, tag="q", name=f"qp{pair}")
                kp = qk.tile([128, 1024], F32R, tag="k", name=f"kp{pair}")
                for jc in range(2):
                    for wt, dst in ((wq, qp), (wk, kp)):
                        qps = psb.tile([128, 512], F32, tag="dot", name="qps")
                        for kk in range(1 if "qkv" in SKIP else 8):
                            nc.tensor.matmul(
                                qps[:],
                                wt[:, kk * 512 + pair * 128 : kk * 512 + (pair + 1) * 128],
                                X2[:, jc * 512 + 2 * kk + 1 : jc * 512 + 2 * kk + 513],
                                start=(kk == 0),
                                stop=(kk == 7),
                            )
                        nc.vector.tensor_copy(dst[:, jc * 512 : (jc + 1) * 512], qps[:])
                return qp, kp

            qkp = emit_qk(0)

            # v^T via transposed fold: lhsT = X2 slice, rhs = wv K-tile
            for i in range(8):
                vps = psb.tile([128, 512], F32, tag="dot", name="vps")
                for kk in range(1 if "qkv" in SKIP else 8):
                    nc.tensor.matmul(
                        vps[:],
                        X2[:, i * 128 + 2 * kk + 1 : i * 128 + 2 * kk + 1 + 128],
                        wv[:, kk * 512 : (kk + 1) * 512],
                        start=(kk == 0),
                        stop=True,
                    )
                nc.vector.tensor_copy(
                    vt[:, i * 520 : i * 520 + 520]
                    .rearrange("p (h c) -> p h c", h=8)[:, :, 0:64],
                    vps[:].rearrange("p (h c) -> p h c", h=8),
                )

            ui = const.tile([128, 4096], F32R, tag="uin")
            for pair in range(4):
                qp, kp = qkp

                h0, h1 = 2 * pair, 2 * pair + 1
                if "attn" in SKIP:
                    nc.vector.tensor_copy(
                        ui[:, pair * 1024 : (pair + 1) * 1024], qp[:]
                    )
                    if pair < 3:
                        qkp = emit_qk(pair + 1)
                    continue
                for jc in range(2):
                    o12 = psa.tile([65, 1024], F32, tag="o12")

                    def emit_out(E_, i_):
                        nc.tensor.matmul(
                            o12[:, 0:512],
                            vt[:, i_ * 520 + h0 * 65 : i_ * 520 + h0 * 65 + 65],
                            E_[:, 0:512],
                            start=(i_ == 0),
                            stop=(i_ == 7),
                        )
                        nc.tensor.matmul(
                            o12[:, 512:1024],
                            vt[:, i_ * 520 + h1 * 65 : i_ * 520 + h1 * 65 + 65],
                            E_[:, 512:1024],
                            start=(i_ == 0),
                            stop=(i_ == 7),
                        )

                    Eprev = None
                    for i in range(8):
                        dps = psd.tile([128, 1024], F32, tag="dot")
                        nc.tensor.matmul(
                            dps[:, 0:512],
                            kp[0:64, i * 128 : (i + 1) * 128],
                            qp[0:64, jc * 512 : (jc + 1) * 512],
                            start=True,
                            stop=True,
                            tile_position=(0, 0),
                        )
                        nc.tensor.matmul(
                            dps[:, 512:1024],
                            kp[64:128, i * 128 : (i + 1) * 128],
                            qp[64:128, jc * 512 : (jc + 1) * 512],
                            start=True,
                            stop=True,
                            tile_position=(64, 0),
                        )
                        E = epool.tile([128, 1024], F32R, tag="E")
                        if "exp" in SKIP:
                            nc.vector.tensor_copy(E[:], dps[:])
                        else:
                            nc.scalar.activation(E[:], dps[:], AF.Exp, bias=expb[:])
                        # software pipeline: out-mms for i-1 emitted after
                        # dot(i) so the in-order PE stream never stalls on
                        # exp(i-1) before starting dot(i)
                        if Eprev is not None:
                            emit_out(Eprev, i - 1)
                        Eprev = E
                    emit_out(Eprev, 7)
                    nc.scalar.activation(lns[:], o12[64:65, :], AF.Ln)
                    nc.scalar.activation(rr[:], lns[:], AF.Exp, scale=-1.0)
                    f1 = psb.tile([64, 512], F32, tag="dot", name="f1")
                    nc.tensor.matmul(
                        f1[:],
                        ones2[:],
                        rr[:, 0:512],
                        start=True,
                        stop=True,
                    )
                    f2 = psb.tile([64, 512], F32, tag="dot", name="f2")
                    nc.tensor.matmul(
                        f2[:],
                        ones2[:],
                        rr[:, 512:1024],
                        start=True,
                        stop=True,
                    )
                    col = pair * 1024 + jc * 512
                    fs1 = qk.tile([64, 512], F32, tag="fs1")
                    nc.vector.tensor_copy(fs1[:], f1[:])
                    nc.vector.tensor_mul(
                        ui[0:64, col : col + 512], o12[0:64, 0:512], fs1[:]
                    )
                    fs2 = qk.tile([64, 512], F32, tag="fs2")
                    nc.vector.tensor_copy(fs2[:], f2[:])
                    nc.vector.tensor_mul(
                        ui[64:128, col : col + 512], o12[0:64, 512:1024], fs2[:]
                    )
                if pair < 3:
                    qkp = emit_qk(pair + 1)

            # unify + residual + instance-norm 1 -> xn
            pre1 = work.tile([64, 1024], F32, tag="pre1")
            for jc in range(2):
                ups = psb.tile([64, 512], F32, tag="dot")
                for pair in range(4):
                    nc.tensor.matmul(
                        ups[:],
                        wu_t[b][:, pair * 64 : (pair + 1) * 64],
                        ui[:, pair * 1024 + jc * 512 : pair * 1024 + (jc + 1) * 512],
                        start=(pair == 0),
                        stop=(pair == 3),
                    )
                nc.vector.tensor_add(
                    pre1[:, jc * 512 : (jc + 1) * 512],
                    ups[:],
                    X2[0:64, 8 + jc * 512 : 8 + (jc + 1) * 512].bitcast(F32),
                )
            xn = work.tile([64, 1024], F32R, tag="xn")
            npt = np_t[b]
            _instance_norm(nc, work, pre1, npt[:, 0:1], npt[:, 1:2], [xn[:]], epst)

            # ffn
            hdn = work.tile([128, 2048], F32R, tag="hdn")
            for mt in range(2):
                for jc in range(2):
                    hps = psb.tile([128, 512], F32, tag="dot")
                    nc.tensor.matmul(
                        hps[:],
                        w1_t[b][:, mt * 128 : (mt + 1) * 128],
                        xn[:, jc * 512 : (jc + 1) * 512],
                        start=True,
                        stop=True,
                    )
                    nc.vector.tensor_relu(
                        hdn[:, mt * 1024 + jc * 512 : mt * 1024 + (jc + 1) * 512],
                        hps[:],
                    )
            pre2 = work.tile([64, 1024], F32, tag="pre2")
            for jc in range(2):
                fps = psb.tile([64, 512], F32, tag="dot")
                for kk in range(2):
                    nc.tensor.matmul(
                        fps[:],
                        w2_t[b][:, kk * 64 : (kk + 1) * 64],
                        hdn[:, kk * 1024 + jc * 512 : kk * 1024 + (jc + 1) * 512],
                        start=(kk == 0),
                        stop=(kk == 1),
                    )
                nc.vector.tensor_add(
                    pre2[:, jc * 512 : (jc + 1) * 512],
                    fps[:],
                    xn[:, jc * 512 : (jc + 1) * 512].bitcast(F32),
                )
            _instance_norm(
                nc,
                work,
                pre2,
                npt[:, 2:3],
                npt[:, 3:4],
                [X2[0:64, 8 : 8 + 1024], X2[64:128, 7 : 7 + 1024]],
                epst,
            )

        # ---- head: sigmoid(wo @ x + bo) = exp(-ln(1 + exp(-z - bo)))
        usb = work.tile([1, 1024], F32, tag="usb")
        for jc in range(2):
            lg = psb.tile([1, 512], F32, tag="dot")
            nc.tensor.matmul(
                lg[:],
                wo_t[:],
                X2[0:64, 8 + jc * 512 : 8 + (jc + 1) * 512],
                start=True,
                stop=True,
            )
            nc.scalar.activation(
                usb[:, jc * 512 : (jc + 1) * 512],
                lg[:],
                AF.Exp,
                bias=negbo[:],
                scale=-1.0,
            )
        v1 = work.tile([1, 1024], F32, tag="v1")
        nc.vector.tensor_single_scalar(v1[:], usb[:], 1.0, op=OP.add)
        w_ = work.tile([1, 1024], F32, tag="w_")
        nc.scalar.activation(w_[:], v1[:], AF.Ln)
        res = work.tile([1, 1024], F32, tag="res")
        nc.scalar.activation(res[:], w_[:], AF.Exp, scale=-1.0)
        nc.sync.dma_start(out_d[:], res[:])


_CACHE = {}


def kernel(x, params):
    x = np.asarray(x, dtype=np.float32)
    prepped = prep_params(params)
    if "nc" not in _CACHE:
        nc = build_nc()
        split_multi_waits(nc)
        _CACHE["nc"] = nc
    nc = _CACHE["nc"]
    in_maps = [{"x": np.ascontiguousarray(x[i]), **prepped} for i in range(B)]
    res = run_bass_kernel_spmd(nc, in_maps, list(range(B)))
    out = np.stack([r["out"] for r in res.results], axis=0)
    return out.astype(np.float32)
